# revision 9
# baseline (speedup 1.0000x reference)
"""CTRGC kernel for Trainium2 (Bass/Tile), 8-core SPMD.

Sharding: core k handles branch b=k//2 and batch half h=k%2 (16 samples).

Per (branch, sample) math (C=64, R=8, T=256, V=25):
  xm  = mean_t x;  x1 = W1 xm;  x2 = W2 xm
  att[r,v,u] = tanh(x1[r,u]-x2[r,v]+b12)
  a = W4 att + (A + b4);  x3 = W3 x + b3
  out[c,t,u] = sum_v a[c,u,v] x3[c,v,t]

Key design points:
  * bf16 I/O and bf16 PE matmuls (1 cyc/row).
  * The c<->v transpose runs as a DVE StreamTranspose with t-PAIRS packed
    into 4-byte units (fp32 bitcast views), halving ST element count.
  * Channels are duplicated across 128 partitions (dup blocks carry a +16
    channel rotation baked into host-side wide weights), so one ST yields
    4 distinct channels per free offset m; block-diagonal lhsT tiles turn
    step 6 into 16 [128x128]x[128x256] matmuls per sample.
  * lhsT / x3sb / a_sb / attw live in persistent ping-pong buffers;
    off-diagonal zeros, v-pad lanes, and the A/ones rows are written once,
    then only live regions are rewritten per sample (zero lhsT rows null
    out pad-lane garbage in the rhs).
  * A + b4 is folded into the a-matmul via two extra att rows (flat A and
    ones) instead of a PSUM-side add — GPSIMD cannot touch PSUM on HW.
  * t-sum via PE identity matmuls (PSUM-accumulated, 18-wide t windows
    sharing one PSUM bank with the x1/x2 matmul outputs).
  * The per-sample PE stream interleaves t-sum, x3, and the previous
    sample's step-6 matmuls (software pipeline) to keep the PE dense;
    x3 runs as 7 quad-v chunks in 2-bank PSUM tiles so evacuation takes
    7 bank-crossing copies instead of 13.
  * x loads are 2-sample batched on SP; out stores go through the Pool
    SWDGE queue; PSUM evacuation is split across ACT/DVE lanes.
"""

import numpy as np

try:
    import concourse  # noqa: F401
except ImportError:  # pragma: no cover
    import sys
    sys.path.insert(0, "/opt/trn_rl_repo")

_CACHE = {}


def _chan(col):
    pb, lo = divmod(col, 32)
    if pb < 2:
        return 32 * pb + lo
    return 32 * (pb - 2) + ((lo + 16) % 32)


_CHANMAP = [_chan(c) for c in range(128)]
_OUT_ORDER = [_chan(32 * i + m) for i in range(4) for m in range(16)]


def _build_nc():
    from concourse import bacc, tile
    from concourse.bass import mybir

    f32 = mybir.dt.float32
    bf16 = mybir.dt.bfloat16
    ALU = mybir.AluOpType
    ACT = mybir.ActivationFunctionType

    nc = bacc.Bacc(None, target_bir_lowering=False)
    x_d = nc.declare_dram_parameter("x", [65, 16, 25, 256], bf16, isOutput=False)
    w3t_d = nc.declare_dram_parameter("w3t", [65, 128], bf16, isOutput=False)
    w12t_d = nc.declare_dram_parameter("w12t", [64, 16], f32, isOutput=False)
    w4t_d = nc.declare_dram_parameter("w4t", [10, 128], bf16, isOutput=False)
    b12_d = nc.declare_dram_parameter("b12", [8, 1], f32, isOutput=False)
    aflat_d = nc.declare_dram_parameter("aflat", [1, 25, 25], bf16,
                                        isOutput=False)
    eye_d = nc.declare_dram_parameter("eye64", [64, 64], bf16, isOutput=False)
    out_d = nc.declare_dram_parameter("out", [128, 16, 16, 256], bf16,
                                      isOutput=True)

    # engine picks for PSUM evacuation (GPSIMD cannot touch PSUM on HW, so
    # only ACT/DVE are legal), interleaved so no lane serializes the
    # producer matmuls (tuned against CoreSim)
    x3_evac = ["act", "dve", "act", "act", "dve", "act", "act"]
    out_evac = ["act", "dve", "act", "dve", "act", "dve", "act", "act"]

    with tile.TileContext(nc) as tc:
        with (
            tc.tile_pool(name="const", bufs=1) as cpool,
            tc.tile_pool(name="xin", bufs=2) as xpool,
            tc.tile_pool(name="x3t", bufs=2) as x3tpool,
            tc.tile_pool(name="aT", bufs=2) as aTpool,
            tc.tile_pool(name="outp", bufs=2) as outpool,
            tc.tile_pool(name="small", bufs=2) as spool,
            tc.tile_pool(name="ps_12", bufs=1, space="PSUM") as ps_12,
            tc.tile_pool(name="ps_a", bufs=1, space="PSUM") as ps_a,
            tc.tile_pool(name="ps_x3", bufs=2, space="PSUM") as ps_x3,
            tc.tile_pool(name="ps_out", bufs=2, space="PSUM") as ps_o,
            # ps banks: ps_12 1 + ps_a 1 + ps_x3 2x2-bank + ps_out 2
        ):
            w3t = cpool.tile([65, 128], bf16)
            nc.sync.dma_start(w3t[:], w3t_d[:])
            w12t = cpool.tile([64, 16], f32)
            nc.sync.dma_start(w12t[:], w12t_d[:])
            w4t = cpool.tile([10, 128], bf16)
            nc.sync.dma_start(w4t[:], w4t_d[:])
            b12 = cpool.tile([8, 1], f32)
            nc.sync.dma_start(b12[:], b12_d[:])
            eye = cpool.tile([64, 64], bf16)
            nc.sync.dma_start(eye[:], eye_d[:])

            # persistent ping-pong state (zeros survive across samples)
            x3sb = [cpool.tile([128, 128, 32], f32, name=f"x3sb{p}",
                               tag=f"x3sb{p}") for p in range(2)]
            lhsT = [cpool.tile([128, 16, 128], bf16, name=f"lhsT{p}",
                               tag=f"lhsT{p}") for p in range(2)]
            asb = [cpool.tile([128, 25, 32], bf16, name=f"asb{p}",
                              tag=f"asb{p}") for p in range(2)]
            # widened att: rows 0-7 tanh (per sample), row 8 = flat A,
            # row 9 = ones; the a-matmul then adds A + b4 directly
            # (w4t rows: [W4.T; ones; b4]) with no PSUM tensor_tensor.
            attw = [cpool.tile([10, 25, 25], bf16, name=f"attw{p}",
                               tag=f"attw{p}") for p in range(2)]
            for p in range(2):
                nc.gpsimd.memset(lhsT[p][:], 0.0)
                nc.gpsimd.memset(x3sb[p][:, :, 25:32], 0.0)
                nc.gpsimd.memset(asb[p][:, :, 25:32], 0.0)
                nc.gpsimd.memset(attw[p][:], 1.0)
                nc.sync.dma_start(attw[p][8:9], aflat_d[:])

            def emit_step6_pair(st, p):
                lh_p, rhs_p, osb, jj = st
                m = 2 * p
                ops = ps_o.tile([128, 2, 256], f32, tag="ops", name="ops")
                for dm in range(2):
                    nc.tensor.matmul(
                        ops[:, dm, :], lh_p[:, m + dm, :],
                        rhs_p[:, :, m + dm, :], start=True, stop=True)
                eng = out_evac[p]
                if eng == "act":
                    nc.scalar.activation(osb[:, jj, m : m + 2, :],
                                         ops[:], ACT.Copy)
                elif eng == "pool":
                    nc.gpsimd.tensor_copy(osb[:, jj, m : m + 2, :], ops[:])
                else:
                    nc.vector.tensor_copy(osb[:, jj, m : m + 2, :], ops[:])

            pend = None           # delayed step-6 stage (software pipeline)
            pend_dma = None       # out DMA to emit after pend's step6
            x2 = None
            outsb = None
            for n in range(16):
                g, j = divmod(n, 2)
                if j == 0:
                    x2 = xpool.tile([65, 2, 25, 256], bf16, tag="x2")
                    if g == 0:
                        nc.sync.dma_start(x2[:, 0], x_d[:, 0])
                        nc.scalar.dma_start(x2[:, 1], x_d[:, 1])
                    else:
                        nc.sync.dma_start(x2[:], x_d[:, 2 * g : 2 * g + 2])
                    outsb = outpool.tile([128, 2, 16, 256], bf16,
                                         tag="outsb")
                xt = x2[:, j]
                x3p = x3sb[n % 2]
                lh = lhsT[n % 2]
                a_sb = asb[n % 2]

                xs12 = ps_12.tile([64, 500], f32, tag="xs12")
                x3bf = x3p[:].bitcast(bf16).rearrange(
                    "c t (v l) -> c t v l", l=2)

                # t-sum windows: 12x20 + 1x16 = 256 (x12 slot borrowed
                # one bank); slot-major PSUM keeps the matmul out contiguous
                def emit_id(s):
                    t0 = 20 * s
                    w = min(t0 + 20, 256) - t0
                    nc.tensor.matmul(
                        xs12[:, 0 : 25 * w].rearrange("p (s v) -> p s v",
                                                      v=25),
                        eye[:],
                        xt[0:64, :, t0 : t0 + w].rearrange("c v t -> c t v"),
                        start=(s == 0), stop=(s == 12))

                def emit_x3(q):
                    v0 = 4 * q
                    nv = 4 if q < 6 else 1
                    x3_ps = ps_x3.tile([128, 4, 256], f32, tag="x3ps",
                                       name="x3_ps")
                    for h in range(0, nv, 2):
                        nh = min(2, nv - h)
                        nc.tensor.matmul(x3_ps[:, h : h + nh, :], w3t[:],
                                         xt[:, v0 + h : v0 + h + nh, :],
                                         start=True, stop=True)
                    src = x3_ps[:, 0:nv, :].rearrange(
                        "c v (t l) -> c v t l", l=2)
                    dst = x3bf[:, :, v0 : v0 + nv, :].rearrange(
                        "c t v l -> c v t l")
                    eng = x3_evac[q]
                    if eng == "act":
                        nc.scalar.activation(dst, src, ACT.Copy)
                    else:
                        nc.vector.tensor_copy(dst, src)

                # --- interleaved PE stream: idmm 0-7, then x3 chunks
                # (front-loaded, so the ST can start early), remaining idmm
                # and the previous sample's step-6 pairs (late, after its
                # lhsT build lands). Keeps the PE dense -> full p-state. ---
                items = [(0.035 * m, "x3", m) for m in range(13)]
                items += [(0.10 + 0.055 * s, "id", s) for s in range(13)]
                if pend is not None:
                    items += [(0.50 + 0.065 * p, "s6", p) for p in range(8)]
                for _, kind, idx in sorted(items):
                    if kind == "x3":
                        emit_x3(idx)
                    elif kind == "id":
                        emit_id(idx)
                    else:
                        emit_step6_pair(pend, idx)
                if pend is not None and pend_dma is not None:
                    eng, args = pend_dma
                    eng.dma_start(*args)
                    pend_dma = None

                xsum = spool.tile([64, 25], f32, tag="xsum")
                nc.vector.tensor_reduce(
                    out=xsum[:],
                    in_=xs12[:].rearrange("p (s v) -> p v s", v=25),
                    axis=mybir.AxisListType.X, op=ALU.add)

                # --- x1/x2 (PSUM slot borrowed from the ps_x3 pool) ---
                x12_tile = ps_x3.tile([128, 4, 256], f32, tag="x3ps",
                                      name="x12_tile")
                x12_ps = x12_tile[0:8, 0, 0:50].rearrange(
                    "p (o v) -> p o v", o=2)
                nc.tensor.matmul(x12_ps[:, 0, :], w12t[:, 0:8], xsum[:],
                                 start=True, stop=True)
                nc.tensor.matmul(x12_ps[:, 1, :], w12t[:, 8:16], xsum[:],
                                 start=True, stop=True)
                x12_sb = spool.tile([8, 2, 25], f32, tag="x12sb")
                nc.scalar.activation(x12_sb[:], x12_ps, ACT.Copy)

                # --- att[r,v,u] = tanh(x1[r,u]-x2[r,v]+b12), bf16 out ---
                attp = spool.tile([8, 25, 25], f32, tag="attp")
                x1b = x12_sb[:, 0:1, :].broadcast_to([8, 25, 25])
                x2b = x12_sb[:, 1:2, :].rearrange(
                    "r o v -> r v o").broadcast_to([8, 25, 25])
                nc.gpsimd.tensor_tensor(attp[:], x1b, x2b, op=ALU.subtract)
                att = attw[n % 2]
                nc.scalar.activation(att[0:8], attp[:], ACT.Tanh,
                                     bias=b12[:], scale=1.0)
                att_f = att[:].rearrange("r v u -> r (v u)")

                # --- a = W4w attw (incl. A + b4) -> asb [c,(u,v32)] bf16 ---
                for ci, (v0, v1) in enumerate(((0, 20), (20, 25))):
                    nf = (v1 - v0) * 25
                    a_ps = ps_a.tile([128, 512], f32,
                                     tag="aps", name=f"a_ps{ci}")
                    nc.tensor.matmul(a_ps[:, 0:nf], w4t[:],
                                     att_f[:, 25 * v0 : 25 * v1],
                                     start=True, stop=True)
                    nc.scalar.activation(
                        a_sb[:, :, v0:v1].rearrange("c u v -> c v u"),
                        a_ps[:, 0:nf].rearrange("c (v u) -> c v u",
                                                v=v1 - v0),
                        ACT.Copy)

                # --- aT4[32i+v, u, c] via StreamTranspose (bf16) ---
                aT4 = aTpool.tile([128, 25, 32], bf16, tag="aT4")
                nc.vector.transpose(aT4[:].rearrange("p u c -> p (u c)"),
                                    a_sb[:].rearrange("p u v -> p (u v)"))

                # --- block-diagonal lhsT (off-diag zeros persist) ---
                for i in range(4):
                    eng = nc.vector if i % 2 == 0 else nc.gpsimd
                    eng.tensor_copy(
                        lh[32 * i : 32 * i + 25, :, 32 * i : 32 * i + 25],
                        aT4[32 * i : 32 * i + 25, :, 0:16].rearrange(
                            "p u c -> p c u"))

                # --- x3T4[32i+v, tp, c] via StreamTranspose (packed) ---
                x3T4 = x3tpool.tile([128, 128, 32], f32, tag="x3T4")
                nc.vector.transpose(
                    x3T4[:].rearrange("p t c -> p (t c)"),
                    x3p[:].rearrange("p t v -> p (t v)"))
                rhs_bf = x3T4[:].bitcast(bf16).rearrange(
                    "p t (c l) -> p t c l", l=2)

                pend = (lh, rhs_bf, outsb, j)
                if j == 1:
                    pend_dma = (nc.gpsimd,
                                (out_d[:, 2 * g : 2 * g + 2], outsb[:]))

            for p in range(8):
                emit_step6_pair(pend, p)
            eng, args = pend_dma
            eng.dma_start(*args)

    nc.compile()
    return nc


def _prep_core(x_half, A_b, W1, B1, W2, B2, W3, B3, W4, B4):
    from ml_dtypes import bfloat16
    f = np.float32
    n = x_half.shape[0]
    xv = np.empty((n, 65, 25, 256), dtype=f)
    xv[:, :64] = x_half.transpose(0, 1, 3, 2)
    xv[:, 64] = 1.0
    xv = np.ascontiguousarray(xv.transpose(1, 0, 2, 3))  # [65, 16, 25, 256]
    w3tfull = np.empty((65, 64), dtype=f)
    w3tfull[:64] = W3.T
    w3tfull[64] = B3
    w3t_wide = np.ascontiguousarray(w3tfull[:, _CHANMAP])
    w12t = np.concatenate([(W1 / 256.0).T, (W2 / 256.0).T], axis=1).astype(f)
    w4t_wide = np.empty((10, 128), dtype=f)
    w4t_wide[0:8] = W4.T[:, _CHANMAP]
    w4t_wide[8] = 1.0
    w4t_wide[9] = B4[_CHANMAP]
    b12 = (B1 - B2).reshape(8, 1).astype(f)
    aflat = np.ascontiguousarray(A_b.T[None, :, :]).astype(f)
    return {
        "x": xv.astype(bfloat16),
        "w3t": w3t_wide.astype(bfloat16),
        "w12t": w12t,
        "w4t": w4t_wide.astype(bfloat16),
        "b12": b12,
        "aflat": aflat.astype(bfloat16),
        "eye64": np.eye(64, dtype=f).astype(bfloat16),
    }


def _unshard_core(out_dev):
    # out_dev: [128, 16, 16, 256] bf16 -> [16, 64, 256, 25] fp32
    o = np.asarray(out_dev).astype(np.float32)
    o = o.reshape(4, 32, 16, 16, 256)[:, :25]       # [i, u, n, m, t]
    o = o.transpose(2, 0, 3, 4, 1).reshape(16, 64, 256, 25)
    full = np.empty_like(o)
    full[:, _OUT_ORDER] = o
    return full


def kernel(**inputs):
    from concourse.bass_utils import run_bass_kernel_spmd

    if "nc" not in _CACHE:
        _CACHE["nc"] = _build_nc()
    nc = _CACHE["nc"]

    A = np.asarray(inputs["A"], dtype=np.float32)
    xs = [np.asarray(inputs[k], dtype=np.float32) for k in ("jo", "bo", "jm", "bm")]
    W = {k: np.asarray(inputs[k], dtype=np.float32)
         for k in ("W1", "B1", "W2", "B2", "W3", "B3", "W4", "B4")}

    in_maps = []
    for k in range(8):
        b, h = k // 2, k % 2
        in_maps.append(_prep_core(
            xs[b][16 * h : 16 * (h + 1)], A[b],
            W["W1"][b], W["B1"][b], W["W2"][b], W["B2"][b],
            W["W3"][b], W["B3"][b], W["W4"][b], W["B4"][b],
        ))

    res = run_bass_kernel_spmd(nc, in_maps, list(range(8))).results

    outs = []
    for b in range(4):
        parts = [_unshard_core(res[2 * b + h]["out"]) for h in range(2)]
        outs.append(np.concatenate(parts, axis=0))
    return tuple(outs)
